# revision 10
# baseline (speedup 1.0000x reference)
"""GQA kernel for Trainium2, 8 NeuronCores (DP over batch x TP over heads).

Problem (hardcoded): B=4, S=1024, EMBED=2048, HEADS=32, GROUPS=8,
GROUP_HEADS=4, HEAD_DIM=64.

Sharding: core c handles batch b = c//2 and TP half m = c%2
(16 heads = 4 groups per core). Host pre-transposes everything and
converts to bf16 so every matmul streams bf16 (1 cyc/row on the PE vs
~2 for fp32r, and FWL hides the LDWEIGHTS):

  qT/kT/vT  [2048, 1024] bf16  (embed-major tokens for one batch)
  wqT       [2048, 1024] bf16  (Wq rows m*1024:(m+1)*1024, T, pre-scaled 1/8)
  wkT/wvT   [2048, 256]  bf16  (Wk/Wv rows m*256:(m+1)*256, transposed)
  wfcT      [1024, 2048] bf16  (Wfc columns m*1024:(m+1)*1024, transposed)
  y         [1024, 2048] f32   partial; host computes y[2b] + y[2b+1] + bfc.

Device pipeline per core (PSUM accumulates f32 everywhere):
  1. K proj: kh psum [2x(go2) x 2x(t2)] [128,512] acc over 16 e-chunks;
     evac to kh_dup[g] [128,1024] bf16 with the group's 64 dims
     duplicated in both partition halves (so score matmuls can run at
     either partition base).
  2. V proj in two 4-bank passes: vh psum [128 tok, 256 vo]; evac to
     vh_aug[kc] [128, 4, 65] bf16 with a trailing ones column (AV then
     emits softmax denominators for free).
  3. Q proj: per ho-chunk psum [128,512] x2 acc over 16 e; evac to
     qh_t[p] [128,1024] bf16 (heads 2p / 2p+1 in partition halves).
  4. Attention per pair p: scores into [128,1024] 2-bank psum, exp on
     ACT (1024-wide amortizes the 352-cyc fixed cost) -> bf16 exp
     tiles; AV accumulates [65,512] psum over kc (stationary vh shared
     by both heads); denominator rows DMA'd to a [2,1024] tile,
     reciprocal_approx_fast, partition_broadcast, DVE mul -> ot bf16.
     Odd head goes through a tmp tile + SBUF DMA into partitions
     64:128 (PE can't write a 65-col result at partition base 64).
  5. FC: y[t] psum [128,512] x4 acc over 8 ho-chunks; evac f32, DMA out.
"""

import numpy as np
import ml_dtypes

import concourse.bass as bass
import concourse.tile as tile
from concourse import bacc, mybir
from concourse.bass_utils import run_bass_kernel_spmd

F32 = mybir.dt.float32
BF16 = mybir.dt.bfloat16
AF = mybir.ActivationFunctionType
NPBF16 = ml_dtypes.bfloat16

B, S, E = 4, 1024, 2048
HEADS_L = 16          # heads per core
GROUPS_L = 4          # groups per core
D = 64                # head dim
P = 128
NE = E // P           # 16 e-chunks
NT = S // P           # 8 token chunks
HO = HEADS_L * D      # 1024 local head-dims
GO = GROUPS_L * D     # 256 local group-dims

_CACHE = {}


def _build():
    nc = bacc.Bacc("TRN2")
    qT = nc.declare_dram_parameter("qT", [E, S], BF16, isOutput=False)
    kT = nc.declare_dram_parameter("kT", [E, S], BF16, isOutput=False)
    vT = nc.declare_dram_parameter("vT", [E, S], BF16, isOutput=False)
    wqT = nc.declare_dram_parameter("wqT", [E, HO], BF16, isOutput=False)
    wkT = nc.declare_dram_parameter("wkT", [E, GO], BF16, isOutput=False)
    wvT = nc.declare_dram_parameter("wvT", [E, GO], BF16, isOutput=False)
    wfcT = nc.declare_dram_parameter("wfcT", [HO, E], BF16, isOutput=False)
    y = nc.declare_dram_parameter("y", [S, E], F32, isOutput=True)

    with tile.TileContext(nc) as tc:
        _body(nc, tc, qT, kT, vT, wqT, wkT, wvT, wfcT, y)
    nc.finalize()
    return nc


def _body(nc, tc, qT, kT, vT, wqT, wkT, wvT, wfcT, y):
    from contextlib import ExitStack
    with ExitStack() as ctx:
        # persistent pools (whole kernel lifetime)
        p_kh = ctx.enter_context(tc.tile_pool(name="kh", bufs=GROUPS_L))
        p_vh = ctx.enter_context(tc.tile_pool(name="vh", bufs=NT))
        p_qh = ctx.enter_context(tc.tile_pool(name="qh", bufs=NT))
        p_ot = ctx.enter_context(tc.tile_pool(name="ot", bufs=NT))
        p_misc = ctx.enter_context(tc.tile_pool(name="misc", bufs=2))

        kh_dup = [p_kh.tile([P, S], BF16, tag="kh", name=f"khdup_{g}")
                  for g in range(GROUPS_L)]
        vh_aug = [p_vh.tile([P, GROUPS_L, D + 1], BF16, tag="vh",
                            name=f"vhaug_{t}") for t in range(NT)]
        qh_t = [p_qh.tile([P, S], BF16, tag="qh", name=f"qh_{t}")
                for t in range(NT)]
        ot_t = [p_ot.tile([P, S], BF16, tag="ot", name=f"ot_{t}")
                for t in range(NT)]

        # warm up the ACT exp table before any real dependency exists
        warm_in = p_misc.tile([1, 8], F32, tag="warm_i")
        warm_out = p_misc.tile([1, 8], F32, tag="warm_o")
        nc.vector.memset(warm_in, 0.0)
        nc.scalar.activation(warm_out, warm_in, AF.Exp)

        with tc.tile_pool(name="xt", bufs=8) as p_xt, \
             tc.tile_pool(name="wkv", bufs=2) as p_wkv:
            # big-strided input tiles: [128, 4, 1024] holds 4 e-chunks
            ktm = [p_xt.tile([P, 4, S], BF16, tag="xt", name=f"ktm_{c}")
                   for c in range(4)]
            vtm = [p_xt.tile([P, 4, S], BF16, tag="xt", name=f"vtm_{c}")
                   for c in range(4)]
            wk_all = p_wkv.tile([P, NE, GO], BF16, tag="wkv", name="wk_all")
            wv_all = p_wkv.tile([P, NE, GO], BF16, tag="wkv", name="wv_all")
            nc.sync.dma_start(
                out=wk_all, in_=wkT[:, :].rearrange("(e p) c -> p e c", p=P))
            for c in range(4):
                nc.sync.dma_start(
                    out=ktm[c],
                    in_=kT[c * 512:(c + 1) * 512, :].rearrange(
                        "(c p) n -> p c n", p=P))
            nc.sync.dma_start(
                out=wv_all, in_=wvT[:, :].rearrange("(e p) c -> p e c", p=P))
            for c in range(4):
                nc.sync.dma_start(
                    out=vtm[c],
                    in_=vT[c * 512:(c + 1) * 512, :].rearrange(
                        "(c p) n -> p c n", p=P))

            # ---- K projection -----------------------------------------
            ps_kv = kv_ctx = tc.tile_pool(name="ps_kv", bufs=2, space="PSUM")
            ps_kv = kv_ctx.__enter__()
            kh_ps = [ps_kv.tile([P, S], F32, tag="kps", bufs=2,
                                name=f"khps_{i}")
                     for i in range(2)]
            for e in range(NE):
                kte = ktm[e // 4][:, e % 4, :]
                for go2 in range(2):
                    lhsT = wk_all[:, e, go2 * P:(go2 + 1) * P]
                    for t2 in range(2):
                        nc.tensor.matmul(
                            kh_ps[go2][:, t2 * 512:(t2 + 1) * 512], lhsT,
                            kte[:, t2 * 512:(t2 + 1) * 512],
                            start=(e == 0), stop=(e == NE - 1))
            for g in range(GROUPS_L):
                go2, half = g // 2, g % 2
                qb = half * D
                nc.vector.tensor_copy(kh_dup[g][qb:qb + D, :],
                                      kh_ps[go2][qb:qb + D, :])
                if half == 0:
                    nc.gpsimd.dma_start(out=kh_dup[g][D:P, :],
                                        in_=kh_dup[g][0:D, :])
                else:
                    nc.gpsimd.dma_start(out=kh_dup[g][0:D, :],
                                        in_=kh_dup[g][D:P, :])

            # ---- V projection (two 4-bank passes) ---------------------
            for half in range(2):
                v_ps = [ps_kv.tile([P, 512], F32, tag="vps", bufs=4,
                               name=f"vps_{half}_{t}")[:, 0:GO]
                        for t in range(4)]
                for e in range(NE):
                    for t in range(4):
                        kc = half * 4 + t
                        nc.tensor.matmul(
                            v_ps[t][:, :],
                            vtm[e // 4][:, e % 4, kc * P:(kc + 1) * P],
                            wv_all[:, e, :],
                            start=(e == 0), stop=(e == NE - 1))
                for t in range(4):
                    kc = half * 4 + t
                    for g in range(GROUPS_L):
                        nc.vector.tensor_copy(
                            vh_aug[kc][:, g, 0:D],
                            v_ps[t][:, g * D:(g + 1) * D])
                    nc.vector.memset(vh_aug[kc][:, :, D:D + 1], 1.0)

            kv_ctx.__exit__(None, None, None)

            # ---- Q projection -----------------------------------------
            with tc.tile_pool(name="wq", bufs=4) as p_wq:
                qtm = [p_xt.tile([P, 4, S], BF16, tag="xt", name=f"qtm_{c}")
                       for c in range(4)]
                wqm = [p_wq.tile([P, 4, HO], BF16, tag="wq", name=f"wqm_{c}")
                       for c in range(4)]
                for c in range(4):
                    nc.sync.dma_start(
                        out=wqm[c],
                        in_=wqT[c * 512:(c + 1) * 512, :].rearrange(
                            "(c p) n -> p c n", p=P))
                    nc.sync.dma_start(
                        out=qtm[c],
                        in_=qT[c * 512:(c + 1) * 512, :].rearrange(
                            "(c p) n -> p c n", p=P))
                q_ctx = tc.tile_pool(name="ps_q", bufs=2, space="PSUM")
                ps_q = q_ctx.__enter__()
                for p in range(NT):
                    q_ps = ps_q.tile([P, S], F32, tag="qps", bufs=2,
                                     name=f"qps_{p}")
                    for e in range(NE):
                        lhsT = wqm[e // 4][:, e % 4, p * P:(p + 1) * P]
                        for t2 in range(2):
                            nc.tensor.matmul(
                                q_ps[:, t2 * 512:(t2 + 1) * 512], lhsT,
                                qtm[e // 4][:, e % 4, t2 * 512:(t2 + 1) * 512],
                                start=(e == 0), stop=(e == NE - 1))
                    nc.vector.tensor_copy(qh_t[p][:, :], q_ps[:, :])
                q_ctx.__exit__(None, None, None)

        # ---- attention + FC ------------------------------------------
        with tc.tile_pool(name="exp", bufs=12) as p_exp, \
             tc.tile_pool(name="sm", bufs=2) as p_sm, \
             tc.tile_pool(name="wfc", bufs=4) as p_wfc, \
             tc.tile_pool(name="ysb", bufs=2) as p_ysb:
            att_ctx = tc.tile_pool(name="ps_att", bufs=2, space="PSUM")
            ps_att = att_ctx.__enter__()
            wfcm = [p_wfc.tile([P, 2, E], BF16, tag="wfc", name=f"wfcm_{c}")
                    for c in range(4)]
            for c in range(4):
                nc.sync.dma_start(
                    out=wfcm[c],
                    in_=wfcT[c * 256:(c + 1) * 256, :].rearrange(
                        "(c p) n -> p c n", p=P))

            for p in range(NT):          # pair p: heads 2p (qb0), 2p+1 (qb64)
                g = p // 2
                exp_t = [p_exp.tile([P, 2 * S], BF16, tag="exp",
                                    name=f"exp_{p}_{kc}", bufs=12)
                         for kc in range(NT)]
                for kc in range(NT):
                    sps = ps_att.tile([P, 2 * S], F32, tag="sps", bufs=1,
                                      name=f"sps_{p}_{kc}")
                    for h2 in range(2):
                        qb = h2 * D
                        lhsT = kh_dup[g][qb:qb + D, kc * P:(kc + 1) * P]
                        for q2 in range(2):
                            nc.tensor.matmul(
                                sps[:, h2 * S + q2 * 512:
                                    h2 * S + (q2 + 1) * 512],
                                lhsT,
                                qh_t[p][qb:qb + D, q2 * 512:(q2 + 1) * 512],
                                start=True, stop=True)
                    nc.scalar.activation(exp_t[kc], sps, AF.Exp)

                av = [ps_att.tile([P, S], F32, tag="avps", bufs=2,
                                  name=f"av_{p}_{h2}") for h2 in range(2)]
                for kc in range(NT):
                    lhsT = vh_aug[kc][:, g, :]
                    for h2 in range(2):
                        for q2 in range(2):
                            nc.tensor.matmul(
                                av[h2][0:D + 1, q2 * 512:(q2 + 1) * 512],
                                lhsT,
                                exp_t[kc][:, h2 * S + q2 * 512:
                                           h2 * S + (q2 + 1) * 512],
                                start=(kc == 0), stop=(kc == NT - 1))

                den_s = p_sm.tile([D + 1, 2, S], F32, tag="dens",
                                  name=f"dens_{p}", bufs=1)
                den = p_sm.tile([2, S], F32, tag="den", name=f"den_{p}",
                                bufs=2)
                recip = p_sm.tile([2, S], F32, tag="recip",
                                  name=f"recip_{p}", bufs=2)
                for h2 in range(2):
                    nc.vector.tensor_copy(den_s[D:D + 1, h2, :],
                                          av[h2][D:D + 1, :])
                nc.gpsimd.dma_start(
                    out=den[:, :].rearrange("p (a b) -> p a b", a=1),
                    in_=den_s[D:D + 1, :, :])
                nc.vector.reciprocal_approx_fast(recip, den)
                recip1 = p_sm.tile([1, S], F32, tag="recip1",
                                   name=f"recip1_{p}", bufs=2)
                nc.gpsimd.dma_start(out=recip1, in_=recip[1:2, :])
                rb = p_sm.tile([D, S], F32, tag="rb", name=f"rb_{p}",
                               bufs=2)
                rb1 = p_sm.tile([D, S], F32, tag="rb1", name=f"rb1_{p}",
                                bufs=2)
                nc.gpsimd.partition_broadcast(rb[0:D, :], recip[0:1, :])
                nc.gpsimd.partition_broadcast(rb1[0:D, :], recip1[0:1, :])
                tmp = p_sm.tile([D, S], BF16, tag="tmp", name=f"tmp_{p}",
                                bufs=2)
                nc.vector.tensor_mul(ot_t[p][0:D, :], av[0][0:D, :],
                                     rb[0:D, :])
                nc.vector.tensor_mul(tmp[0:D, :], av[1][0:D, :],
                                     rb1[0:D, :])
                nc.gpsimd.dma_start(out=ot_t[p][D:P, :], in_=tmp[0:D, :])

            # ---- FC --------------------------------------------------
            att_ctx.__exit__(None, None, None)
            ps_y = y_ctx = tc.tile_pool(name="ps_y", bufs=2, space="PSUM")
            ps_y = y_ctx.__enter__()
            for t in range(NT):
                y_ps = [ps_y.tile([P, S], F32, tag="yps", bufs=2,
                                  name=f"yps_{t}_{r2}")
                        for r2 in range(2)]
                for i in range(NT):
                    lhsT = ot_t[i][:, t * P:(t + 1) * P]
                    for r2 in range(2):
                        for f2 in range(2):
                            nc.tensor.matmul(
                                y_ps[r2][:, f2 * 512:(f2 + 1) * 512], lhsT,
                                wfcm[i // 2][:, i % 2,
                                             r2 * S + f2 * 512:
                                             r2 * S + (f2 + 1) * 512],
                                start=(i == 0), stop=(i == NT - 1))
                y_sb = p_ysb.tile([P, E], F32, tag="ysb", name=f"ysb_{t}")
                for r2 in range(2):
                    nc.vector.tensor_copy(y_sb[:, r2 * S:(r2 + 1) * S],
                                          y_ps[r2][:, :])
                nc.sync.dma_start(out=y[t * P:(t + 1) * P, :], in_=y_sb)
            y_ctx.__exit__(None, None, None)


def _get_nc():
    if "nc" not in _CACHE:
        _CACHE["nc"] = _build()
    return _CACHE["nc"]


def _make_in_maps(q, k, v, Wq, Wk, Wv, Wfc):
    q = np.asarray(q, np.float32)
    k = np.asarray(k, np.float32)
    v = np.asarray(v, np.float32)
    Wq = np.asarray(Wq, np.float32)
    Wk = np.asarray(Wk, np.float32)
    Wv = np.asarray(Wv, np.float32)
    Wfc = np.asarray(Wfc, np.float32)

    qTb = [np.ascontiguousarray(q[b].T).astype(NPBF16) for b in range(B)]
    kTb = [np.ascontiguousarray(k[b].T).astype(NPBF16) for b in range(B)]
    vTb = [np.ascontiguousarray(v[b].T).astype(NPBF16) for b in range(B)]
    wqTm = [np.ascontiguousarray((Wq[m * HO:(m + 1) * HO, :] / 8.0).T).astype(NPBF16)
            for m in range(2)]
    wkTm = [np.ascontiguousarray(Wk[m * GO:(m + 1) * GO, :].T).astype(NPBF16)
            for m in range(2)]
    wvTm = [np.ascontiguousarray(Wv[m * GO:(m + 1) * GO, :].T).astype(NPBF16)
            for m in range(2)]
    wfcTm = [np.ascontiguousarray(Wfc[:, m * HO:(m + 1) * HO].T).astype(NPBF16)
             for m in range(2)]

    in_maps = []
    for c in range(8):
        b, m = c // 2, c % 2
        in_maps.append({
            "qT": qTb[b], "kT": kTb[b], "vT": vTb[b],
            "wqT": wqTm[m], "wkT": wkTm[m], "wvT": wvTm[m],
            "wfcT": wfcTm[m],
        })
    return in_maps


def kernel(q, k, v, Wq, Wk, Wv, Wfc, bfc):
    bfc = np.asarray(bfc, np.float32)
    nc = _get_nc()
    in_maps = _make_in_maps(q, k, v, Wq, Wk, Wv, Wfc)
    res = run_bass_kernel_spmd(nc, in_maps, list(range(8)))
    out = np.empty((B, S, E), np.float32)
    for b in range(B):
        out[b] = res.results[2 * b]["y"] + res.results[2 * b + 1]["y"] + bfc
    return out
